# revision 6
# baseline (speedup 1.0000x reference)
"""Diffeomorphic transform (scaling-and-squaring, TIME_STEP=7) on 8 TRN2
NeuronCores.

One device program runs all 7 squaring steps; between steps the 8 z-shards
are exchanged on-device with an AllGather into a ping-pong pair of
full-volume HBM buffers, so the host stages the flow exactly once and reads
the result once.

Layout: channel-minor volume vol[(((z*H)+y)*W+x)*3 + c] padded with PAD
zeros so unclamped y+1 / x+1 corner reads at the upper edge stay in bounds
(their lerp weight is exactly 0, and the fetched pad values are finite).

Per chunk of 128 x M voxels (partition p holds M consecutive voxels =
M/W full x-rows, so x starts at 0 and (z, y-base) are constant per
partition):
  Pool: indirect load of this core's own shard slice (fch); 4*M indirect
        row gathers (one per voxel row per (z,y) corner, 128 offsets each,
        6 contiguous floats per offset = (x0,x0+1) x 3 channels); the
        store. The y+1 corner reuses the y offsets via element_offset.
  DVE:  coords from f*scale + base grid (base grid built from per-chunk
        per-partition (z,y) constants and a static x/y ramp — no HBM
        grid), clip, rounding-mode-agnostic floor, lerp weights, corner
        offsets; then trilinear combine + add f, written
        channel-interleaved so shards concatenate directly.

The runner is a trimmed copy of concourse.bass2jax.run_bass_via_pjrt that
reads the sharded output back with a single device-to-host copy and caches
the jitted callable; the JAX persistent compilation cache is enabled so a
fresh process skips the BIR->NEFF compile.
"""
import os
import sys
sys.path.insert(0, '/opt/trn_rl_repo')
from contextlib import ExitStack
import numpy as np

os.environ.setdefault("JAX_COMPILATION_CACHE_DIR", "/root/.cache/jax_bass")

import jax
jax.config.update("jax_compilation_cache_dir",
                  os.environ["JAX_COMPILATION_CACHE_DIR"])
jax.config.update("jax_persistent_cache_min_compile_time_secs", 0.0)
jax.config.update("jax_persistent_cache_min_entry_size_bytes", 0)

import jax.numpy as jnp
from jax.experimental.shard_map import shard_map
from jax.sharding import Mesh, NamedSharding, PartitionSpec
from jax.interpreters import mlir as _jmlir
from jax._src.interpreters.mlir import custom_call as _mlir_custom_call

import concourse.bass as bass
from concourse import mybir
from concourse.bass2jax import (
    bass_effect,
    install_neuronx_cc_hook,
    partition_id_tensor,
)

# Primitive that emits the same "bass_exec" custom call as
# concourse.bass2jax, but from a pre-serialized backend_config — so warm
# processes skip building the 134k-instruction Bass program entirely.
_cached_exec_p = jax.extend.core.Primitive("bass_exec")
_cached_exec_p.multiple_results = True


@_cached_exec_p.def_effectful_abstract_eval
def _cached_abstract_eval(*_, out_avals, **__):
    return out_avals, {bass_effect}


def _cached_lowering(ctx, *in_nodes, out_avals, backend_config,
                     has_collectives):
    result_types = [_jmlir.aval_to_ir_type(a) for a in ctx.avals_out]
    operand_layouts = [list(reversed(range(len(a.shape))))
                       for a in ctx.avals_in]
    result_layouts = [list(reversed(range(len(a.shape))))
                      for a in ctx.avals_out]
    fa = {}
    if has_collectives:
        fa["has_collectives"] = _jmlir.ir.StringAttr.get("1")
    return _mlir_custom_call(
        "bass_exec",
        operands=in_nodes,
        result_types=result_types,
        operand_layouts=operand_layouts,
        result_layouts=result_layouts,
        backend_config=backend_config,
        extra_attributes={
            "mhlo.frontend_attributes": _jmlir.ir.DictAttr.get(fa)
        },
    ).results


try:
    _jmlir.register_lowering(_cached_exec_p, _cached_lowering,
                             platform="neuron")
except NotImplementedError:
    pass

_BUILD_TAG = "diffeo-v4.4"
_PERSIST_DIR = os.path.expanduser("~/.cache/bass_diffeo")

F32 = mybir.dt.float32
I32 = mybir.dt.int32
PAD = 512
Alu = mybir.AluOpType

_CACHE = {}


def build(D, H, W, n_cores, M, steps):
    C = 3
    N = D * H * W
    NN = N * C
    WC = W * C
    shard_d = D // n_cores
    assert shard_d * n_cores == D
    vox_nc = shard_d * H * W          # voxels per core
    S = vox_nc * C                    # elements per core shard
    vox_ck = 128 * M                  # voxels per chunk
    n_chunks = vox_nc // vox_ck
    assert n_chunks * vox_ck == vox_nc
    assert M % W == 0                 # partition rows start at x=0
    assert (H * W) % M == 0           # rows never cross a z-plane (bgz/bgy
                                      # are per-partition constants)
    MC = M * C
    GS = 32 + 64 * M                  # per-chunk gsem: load 16 + 4M*16 + store 16
    AFTER_G = 16 + 64 * M
    sx, sy, sz = 0.5 * (W - 1), 0.5 * (H - 1), 0.5 * (D - 1)
    groups = [list(range(n_cores))]

    nc = bass.Bass(num_devices=n_cores, detect_race_conditions=False,
                   disable_frame_to_traceback=True)
    fsh_in = nc.dram_tensor("fsh", [S, 1], F32, kind="ExternalInput")
    offs_in = nc.dram_tensor("offs", [128, n_chunks], I32, kind="ExternalInput")
    xy_in = nc.dram_tensor("xyrow", [128, 2 * M], F32, kind="ExternalInput")
    zy_in = nc.dram_tensor("bgzy", [128, 2 * n_chunks], F32, kind="ExternalInput")
    outp = nc.dram_tensor("outp", [S, 1], F32, kind="ExternalOutput")
    vol0 = nc.dram_tensor("vol0", [NN + PAD, 1], F32, addr_space="Shared")
    vol1 = nc.dram_tensor("vol1", [NN + PAD, 1], F32, addr_space="Shared")
    shardbuf = nc.dram_tensor("shardbuf", [S, 1], F32)

    with ExitStack() as ctx:
        def sb(nm, shape, dt):
            return ctx.enter_context(nc.sbuf_tensor(nm, shape, dt))
        fch = sb("fch", [128, MC], F32)
        out3 = sb("outt3", [128, MC], F32)
        xyrow = sb("xyrow_sb", [128, 2 * M], F32)
        bgzy = sb("bgzy_sb", [128, 2 * n_chunks], F32)
        tz, ty, tx = sb("tz", [128, M], F32), sb("tty", [128, M], F32), sb("ttx", [128, M], F32)
        ti = sb("tti", [128, M], I32)
        zf, yf, xf = sb("zf", [128, M], F32), sb("yf", [128, M], F32), sb("xff", [128, M], F32)
        wz, wy, wx = sb("wz", [128, M], F32), sb("wy", [128, M], F32), sb("wx", [128, M], F32)
        mm = sb("mmm", [128, M], F32)
        tc = sb("tcc", [128, M], F32)
        x3 = sb("x33", [128, M], F32)
        uu = sb("uuu", [128, M], F32)
        z1f = sb("z1f", [128, M], F32)
        o0, o1 = sb("oo0", [128, M], F32), sb("oo1", [128, M], F32)
        i00, i10 = sb("i00", [128, M], I32), sb("i10", [128, M], I32)
        g00, g01 = sb("g00", [128, M * 6], F32), sb("g01", [128, M * 6], F32)
        g10, g11 = sb("g10", [128, M * 6], F32), sb("g11", [128, M * 6], F32)
        offs_sb = sb("offs_sb", [128, n_chunks], I32)
        ztile = sb("ztile", [128, PAD // 128], F32)
        gsems = [ctx.enter_context(nc.semaphore(f"gsem{i}"))
                 for i in range(n_chunks)]
        csem = ctx.enter_context(nc.semaphore("csem"))
        bsem = ctx.enter_context(nc.semaphore("bsem"))
        msem = ctx.enter_context(nc.semaphore("msem"))
        ccsem = ctx.enter_context(nc.semaphore("ccsem"))
        block = ctx.enter_context(nc.Block())

        xrow = xyrow[:, 0:M]
        yrow = xyrow[:, M:2 * M]

        @block.gpsimd
        def _(gp):
            # bootstrap: bounce own shard to Internal, zero vol pads, load
            # the small constant tables, AllGather shards -> vol0
            gp.dma_start(out=shardbuf[:, :], in_=fsh_in[:, :]).then_inc(bsem, 16)
            gp.memset(ztile[:], 0.0).then_inc(msem, 1)
            gp.wait_ge(msem, 1)
            gp.wait_ge(bsem, 16)
            gp.dma_start(
                out=vol0[NN:NN + PAD, 0].rearrange("(p m) -> p m", p=128),
                in_=ztile[:, :]).then_inc(bsem, 16)
            gp.dma_start(
                out=vol1[NN:NN + PAD, 0].rearrange("(p m) -> p m", p=128),
                in_=ztile[:, :]).then_inc(bsem, 16)
            gp.dma_start(out=offs_sb[:, :], in_=offs_in[:, :]).then_inc(bsem, 16)
            gp.dma_start(out=xyrow[:, :], in_=xy_in[:, :]).then_inc(bsem, 16)
            gp.dma_start(out=bgzy[:, :], in_=zy_in[:, :]).then_inc(bsem, 16)
            gp.wait_ge(bsem, 96)
            gp.collective_compute(
                "AllGather", Alu.bypass, replica_groups=groups,
                ins=[shardbuf[:, :].opt()],
                outs=[vol0[0:NN, :].opt()],
            ).then_inc(ccsem, 1)

            gval = [0] * n_chunks
            for s in range(steps):
                vol = vol0 if s % 2 == 0 else vol1
                nxt = vol1 if s % 2 == 0 else vol0
                gp.wait_ge(ccsem, s + 1)
                for ck in range(n_chunks):
                    t = s * n_chunks + ck
                    gsem = gsems[ck]
                    gb = gval[ck]
                    off = ck * vox_ck * C
                    gp.indirect_dma_start(
                        out=fch[:, :], out_offset=None, in_=vol[:, :],
                        in_offset=bass.IndirectOffsetOnAxis(
                            ap=offs_sb[:, ck:ck + 1], axis=0),
                    ).then_inc(gsem, 16)
                    gp.wait_ge(gsem, gb + 16)
                    gp.sem_inc(csem, 1)          # 4t+1: load visible
                    gp.wait_ge(csem, 4 * t + 2)  # phase A done
                    # the HW consumes one offset per partition per indirect
                    # DMA, so each voxel row needs its own instruction per
                    # (z,y) corner; the y+1 row reuses the same offsets
                    # shifted by one x-row via element_offset.
                    for jv in range(M):
                        for gt, it_, eo in ((g00, i00, 0), (g01, i00, WC),
                                            (g10, i10, 0), (g11, i10, WC)):
                            gp.indirect_dma_start(
                                out=gt[:, jv * 6:(jv + 1) * 6],
                                out_offset=None, in_=vol[:, :],
                                in_offset=bass.IndirectOffsetOnAxis(
                                    ap=it_[:, jv:jv + 1], axis=0),
                                element_offset=eo,
                            ).then_inc(gsem, 16)
                    gp.sem_inc(csem, 1)          # 4t+3: gathers issued
                    gp.wait_ge(csem, 4 * t + 4)  # combine done
                    dest = outp if s == steps - 1 else shardbuf
                    gp.dma_start(
                        out=dest[off:off + vox_ck * C, 0]
                            .rearrange("(p m) -> p m", p=128),
                        in_=out3[:, :]).then_inc(gsem, 16)
                    gp.wait_ge(gsem, gb + GS)    # chunk fully drained
                    gval[ck] += GS
                if s < steps - 1:
                    gp.collective_compute(
                        "AllGather", Alu.bypass, replica_groups=groups,
                        ins=[shardbuf[:, :].opt()],
                        outs=[nxt[0:NN, :].opt()],
                    ).then_inc(ccsem, 1)

        @block.vector
        def _(ve):
            def f3(ch):
                return fch[:, ch::C]
            def o3(ch):
                return out3[:, ch::C]
            gsl = {(0, 0): g00, (0, 1): g01, (1, 0): g10, (1, 1): g11}

            gval = [0] * n_chunks
            for s in range(steps):
                for ck in range(n_chunks):
                    t = s * n_chunks + ck
                    gsem = gsems[ck]
                    gb = gval[ck]
                    ve.wait_ge(csem, 4 * t + 1)

                    bgz = bgzy[:, ck:ck + 1]
                    bgy = bgzy[:, n_chunks + ck:n_chunks + ck + 1]

                    def floorw(f, w):
                        # rounding-mode-agnostic floor of tc (>= 0): convert,
                        # convert back, subtract 1 where the cast overshot
                        ve.tensor_scalar(out=ti[:], in0=tc[:], scalar1=1,
                                         scalar2=None, op0=Alu.mult)
                        ve.tensor_scalar(out=f[:], in0=ti[:], scalar1=1,
                                         scalar2=None, op0=Alu.mult)
                        ve.tensor_tensor(out=uu[:], in0=f[:], in1=tc[:],
                                         op=Alu.is_gt)
                        ve.tensor_tensor(out=f[:], in0=f[:], in1=uu[:],
                                         op=Alu.subtract)
                        ve.tensor_tensor(out=w[:], in0=tc[:], in1=f[:],
                                         op=Alu.subtract)

                    # z: coord = f*sz + bgz  (bgz constant per partition)
                    ve.tensor_scalar(out=tz[:], in0=f3(0), scalar1=float(sz),
                                     scalar2=bgz, op0=Alu.mult, op1=Alu.add)
                    ve.tensor_scalar(out=tc[:], in0=tz[:], scalar1=0.0,
                                     scalar2=float(D - 1), op0=Alu.max,
                                     op1=Alu.min)
                    floorw(zf, wz)
                    # y: coord = (f*sy + yrow) + bgy — add the large integer
                    # part last so the single rounding matches the reference
                    ve.tensor_scalar(out=uu[:], in0=f3(1), scalar1=float(sy),
                                     scalar2=None, op0=Alu.mult)
                    ve.tensor_tensor(out=tz[:], in0=uu[:], in1=yrow,
                                     op=Alu.add)
                    ve.tensor_scalar(out=uu[:], in0=tz[:], scalar1=bgy,
                                     scalar2=None, op0=Alu.add)
                    ve.tensor_scalar(out=tc[:], in0=uu[:], scalar1=0.0,
                                     scalar2=float(H - 1), op0=Alu.max,
                                     op1=Alu.min)
                    floorw(yf, wy)
                    # x: coord = f*sx + xrow
                    ve.tensor_scalar(out=uu[:], in0=f3(2), scalar1=float(sx),
                                     scalar2=None, op0=Alu.mult)
                    ve.tensor_tensor(out=tz[:], in0=uu[:], in1=xrow,
                                     op=Alu.add)
                    ve.tensor_scalar(out=tc[:], in0=tz[:], scalar1=0.0,
                                     scalar2=float(W - 1), op0=Alu.max,
                                     op1=Alu.min)
                    floorw(xf, wx)

                    ve.tensor_scalar(out=z1f[:], in0=zf[:], scalar1=1.0,
                                     scalar2=float(D - 1), op0=Alu.add,
                                     op1=Alu.min)
                    ve.tensor_scalar(out=x3[:], in0=xf[:], scalar1=3.0,
                                     scalar2=None, op0=Alu.mult)
                    ve.tensor_scalar(out=uu[:], in0=zf[:], scalar1=float(H),
                                     scalar2=None, op0=Alu.mult)
                    ve.tensor_tensor(out=tz[:], in0=uu[:], in1=yf[:], op=Alu.add)
                    ve.tensor_scalar(out=uu[:], in0=tz[:], scalar1=float(WC),
                                     scalar2=None, op0=Alu.mult)
                    ve.tensor_tensor(out=o0[:], in0=uu[:], in1=x3[:], op=Alu.add)
                    ve.tensor_scalar(out=uu[:], in0=z1f[:], scalar1=float(H),
                                     scalar2=None, op0=Alu.mult)
                    ve.tensor_tensor(out=tz[:], in0=uu[:], in1=yf[:], op=Alu.add)
                    ve.tensor_scalar(out=uu[:], in0=tz[:], scalar1=float(WC),
                                     scalar2=None, op0=Alu.mult)
                    ve.tensor_tensor(out=o1[:], in0=uu[:], in1=x3[:], op=Alu.add)
                    ve.tensor_scalar(out=i00[:], in0=o0[:], scalar1=1.0,
                                     scalar2=None, op0=Alu.mult)
                    ve.tensor_scalar(out=i10[:], in0=o1[:], scalar1=1.0,
                                     scalar2=None,
                                     op0=Alu.mult).then_inc(csem, 1)  # 4t+2

                    ve.wait_ge(gsem, gb + AFTER_G)
                    last = None
                    for k in range(C):
                        for sN, dst in ((0, o0), (1, o1)):
                            ylp = {}
                            for yy in (0, 1):
                                a = gsl[(sN, yy)][:, k::6]
                                b = gsl[(sN, yy)][:, 3 + k::6]
                                ve.tensor_tensor(out=tz[:], in0=b, in1=a,
                                                 op=Alu.subtract)
                                ve.tensor_tensor(out=uu[:], in0=tz[:],
                                                 in1=wx[:], op=Alu.mult)
                                t_xl = ty if yy == 0 else tx
                                ve.tensor_tensor(out=t_xl[:], in0=uu[:],
                                                 in1=a, op=Alu.add)
                                ylp[yy] = t_xl
                            ve.tensor_tensor(out=mm[:], in0=ylp[1][:],
                                             in1=ylp[0][:], op=Alu.subtract)
                            ve.tensor_tensor(out=uu[:], in0=mm[:], in1=wy[:],
                                             op=Alu.mult)
                            ve.tensor_tensor(out=dst[:], in0=uu[:],
                                             in1=ylp[0][:], op=Alu.add)
                        ve.tensor_tensor(out=mm[:], in0=o1[:], in1=o0[:],
                                         op=Alu.subtract)
                        ve.tensor_tensor(out=uu[:], in0=mm[:], in1=wz[:],
                                         op=Alu.mult)
                        ve.tensor_tensor(out=mm[:], in0=uu[:], in1=o0[:],
                                         op=Alu.add)
                        last = ve.tensor_tensor(out=o3(k), in0=mm[:],
                                                in1=f3(k), op=Alu.add)
                    last.then_inc(csem, 1)       # 4t+4: combine done
                    gval[ck] += GS
    return nc


def make_inputs(flow, D, H, W, n_cores, M, steps):
    C = 3
    HW = H * W
    shard_d = D // n_cores
    vox_nc = shard_d * HW
    S = vox_nc * C
    vox_ck = 128 * M
    n_chunks = vox_nc // vox_ck
    f = (np.asarray(flow[0]).astype(np.float32) / float(2 ** steps))
    fcm = np.ascontiguousarray(f.transpose(1, 2, 3, 0)).reshape(-1)

    jj = np.arange(M, dtype=np.float32)
    xy = np.empty((128, 2 * M), np.float32)
    xy[:, 0:M] = (jj % W)[None, :]
    xy[:, M:2 * M] = (jj // W)[None, :]

    pp = np.arange(128, dtype=np.int64)[:, None]
    cks = np.arange(n_chunks, dtype=np.int64)[None, :]
    vstart = cks * vox_ck + pp * M           # local voxel start

    in_maps = []
    for k in range(n_cores):
        lo = k * S
        gstart = vstart + k * vox_nc         # global voxel index
        zy = np.empty((128, 2 * n_chunks), np.float32)
        zy[:, 0:n_chunks] = (gstart // HW).astype(np.float32)
        zy[:, n_chunks:] = ((gstart % HW) // W).astype(np.float32)
        offs = (lo + vstart * C).astype(np.int32)
        in_maps.append({
            "fsh": np.ascontiguousarray(fcm[lo:lo + S]).reshape(-1, 1),
            "offs": offs,
            "xyrow": xy,
            "bgzy": zy,
        })
    return in_maps


def assemble(parts, D, H, W):
    full = np.concatenate(parts).reshape(D, H, W, 3)
    return np.ascontiguousarray(full.transpose(3, 0, 1, 2)[None])


_RUNNER = {}


def _extract_meta(nc):
    """Serialize everything the jit needs from a built Bass program."""
    import base64
    import orjson
    import zstandard
    partition_name = (nc.partition_id_tensor.name
                      if nc.partition_id_tensor else None)
    in_names, out_names, zero_shapes = [], [], []
    for alloc in nc.m.functions[0].allocations:
        if not isinstance(alloc, mybir.MemoryLocationSet):
            continue
        name = alloc.memorylocations[0].name
        if alloc.kind == "ExternalInput":
            if name != partition_name:
                in_names.append(name)
        elif alloc.kind == "ExternalOutput":
            assert alloc.tensor_shape is not None and alloc.dtype is not None
            out_names.append(name)
            zero_shapes.append((tuple(alloc.tensor_shape),
                                np.dtype(mybir.dt.np(alloc.dtype))))
    all_in_names = (in_names + out_names
                    + ([partition_name] if partition_name else []))
    compressed = zstandard.ZstdCompressor().compress(nc.to_json_bytes())
    config = {
        "ant_bir": base64.standard_b64encode(compressed).decode(),
        "in_names": all_in_names,
        "out_names": out_names,
        "arch": nc.m.arch,
    }
    backend_config = base64.standard_b64encode(
        orjson.dumps(config, option=orjson.OPT_INDENT_2)).decode()
    return {
        "tag": _BUILD_TAG,
        "backend_config": backend_config,
        "in_names": in_names,
        "out_names": out_names,
        "zero_shapes": zero_shapes,
        "has_partition_id": partition_name is not None,
        "has_collectives": bool(nc.has_collectives),
    }


def _meta_path(key):
    return os.path.join(_PERSIST_DIR,
                        "meta_" + "_".join(str(k) for k in key) + ".pkl")


def _load_meta(key):
    import pickle
    try:
        with open(_meta_path(key), "rb") as f:
            meta = pickle.load(f)
        if meta.get("tag") != _BUILD_TAG:
            return None
        return meta
    except Exception:
        return None


def _save_meta(key, meta):
    import pickle
    try:
        os.makedirs(_PERSIST_DIR, exist_ok=True)
        tmp = _meta_path(key) + ".tmp"
        with open(tmp, "wb") as f:
            pickle.dump(meta, f)
        os.replace(tmp, _meta_path(key))
    except Exception:
        pass


def _make_runner(meta, n_cores):
    """One jitted shard_map built from serialized program metadata, with a
    single device-to-host copy of the sharded output."""
    install_neuronx_cc_hook()
    in_names = meta["in_names"]
    out_names = meta["out_names"]
    zero_shapes = meta["zero_shapes"]
    out_avals = [jax.core.ShapedArray(s, d) for s, d in zero_shapes]
    n_params = len(in_names)
    n_outs = len(out_avals)
    donate = tuple(range(n_params, n_params + n_outs))
    backend_config = meta["backend_config"]
    has_collectives = meta["has_collectives"]
    has_pid = meta["has_partition_id"]

    def _body(*args):
        operands = list(args)
        if has_pid:
            operands.append(partition_id_tensor())
        outs = _cached_exec_p.bind(
            *operands,
            out_avals=tuple(out_avals),
            backend_config=backend_config,
            has_collectives=has_collectives,
        )
        return tuple(outs)

    devices = jax.devices()[:n_cores]
    assert len(devices) == n_cores
    mesh = Mesh(np.asarray(devices), ("core",))
    in_specs = (PartitionSpec("core"),) * (n_params + n_outs)
    out_specs = (PartitionSpec("core"),) * n_outs
    sharded = jax.jit(
        shard_map(_body, mesh=mesh, in_specs=in_specs, out_specs=out_specs,
                  check_rep=False),
        donate_argnums=donate, keep_unused=True,
    )
    shd = NamedSharding(mesh, PartitionSpec("core"))
    zeros_fn = jax.jit(
        lambda: tuple(jnp.zeros((n_cores * s[0], *s[1:]), d)
                      for s, d in zero_shapes),
        out_shardings=shd)
    return sharded, zeros_fn, in_names, out_names, out_avals, zero_shapes


def _sharding(n_cores):
    mesh = Mesh(np.asarray(jax.devices()[:n_cores]), ("core",))
    return NamedSharding(mesh, PartitionSpec("core"))


def run(flow, D, H, W, n_cores, M, steps):
    # Stage the inputs to the devices asynchronously FIRST, so the 59 MB
    # upload streams while any program build / jit tracing runs.
    in_maps = make_inputs(flow, D, H, W, n_cores, M, steps)
    in_name_order = ["fsh", "offs", "xyrow", "bgzy"]
    shd = _sharding(n_cores)
    dev_in = {
        name: jax.device_put(
            np.concatenate([in_maps[c][name] for c in range(n_cores)], axis=0),
            shd)
        for name in in_name_order
    }

    key = (D, H, W, n_cores, M, steps)
    if key not in _RUNNER:
        meta = _load_meta(key)
        if meta is None:
            nc = build(D, H, W, n_cores, M, steps)
            meta = _extract_meta(nc)
            _save_meta(key, meta)
            del nc
        _RUNNER[key] = _make_runner(meta, n_cores)
    sharded, zeros_fn, in_names, out_names, out_avals, zero_shapes = _RUNNER[key]

    concat_zeros = zeros_fn()        # donated output buffers, created on-device
    out_arrs = sharded(*[dev_in[name] for name in in_names], *concat_zeros)
    # parallel per-shard D2H (run_bass_via_pjrt copies the global once per
    # core, multiplying the transfer 8x)
    from concurrent.futures import ThreadPoolExecutor
    shards = sorted(out_arrs[0].addressable_shards,
                    key=lambda s: s.index[0].start or 0)
    with ThreadPoolExecutor(len(shards)) as ex:
        parts = list(ex.map(lambda s: np.asarray(s.data).reshape(-1), shards))
    return assemble(parts, D, H, W)


def kernel(flow):
    return run(flow, 160, 192, 160, n_cores=8, M=480, steps=7)


def measure_exec_ns(flow, iters=3):
    """Measure the device execution time of the full 7-step computation with
    inputs already resident in HBM (what neuron-profile would report)."""
    import time
    D, H, W, n_cores, M, steps = 160, 192, 160, 8, 480, 7
    kernel(flow)  # ensure compiled + runner cached
    in_maps = make_inputs(flow, D, H, W, n_cores, M, steps)
    shd = _sharding(n_cores)
    dev_in = {
        name: jax.device_put(
            np.concatenate([in_maps[c][name] for c in range(n_cores)], axis=0),
            shd)
        for name in ["fsh", "offs", "xyrow", "bgzy"]
    }
    jax.block_until_ready(list(dev_in.values()))
    sharded, zeros_fn, in_names, _, _, _ = _RUNNER[(D, H, W, n_cores, M, steps)]
    best = None
    for _ in range(iters):
        cz = zeros_fn()
        jax.block_until_ready(cz)
        t0 = time.perf_counter()
        out = sharded(*[dev_in[n] for n in in_names], *cz)
        jax.block_until_ready(out)
        dt = time.perf_counter() - t0
        best = dt if best is None or dt < best else best
    return int(best * 1e9)
